# revision 7
# baseline (speedup 1.0000x reference)
"""Trainium2 Bass kernel for a 2-layer GAT (nn_GAT_Module_90623809945643).

Strategy (8 NeuronCores, SPMD):
  - Nodes are partitioned across the 8 cores by global id (6250 each); each
    core owns the edges whose dst it owns, so edge-softmax and segment-sum
    stay local (dst-owner bucketing).
  - Per layer, every core builds the full per-node feature table
    [h (bf16, 256) | el (f32, 4) | er (f32, 4) | pad] (768B rows) from
    replicated x (layer 1) / the all-gathered layer-1 output (layer 2) with
    TensorE matmuls; attn_l/attn_r are folded into the weight matrix on the
    host (el = x @ (W_h @ attn_l_h)), so el/er are matmul outputs.
  - Edge phase: nodes-on-partitions layout. The host sorts each core's nodes
    by in-degree and packs each node's incoming edges into fixed-width slot
    grids (degree bucketing); h[src]/el[src] rows are fetched with dma_gather
    (int16 indices, table split at the 32768-row boundary), er comes
    per-partition via indirect DMA. The weighted segment-sum runs as
    broadcast-exp (ACT) + multiply + pairwise-tree reduction (DVE); the
    softmax division happens once per node after aggregation (the reference's
    max-subtraction cancels exactly and is skipped; scores are O(1) so exp
    cannot overflow).
  - One AllGather (bf16, transposed layer-1 output) between the layers.

The host performs index-only preprocessing (bucketing, padding, index
streams); all floating-point work on x happens on the device.
"""

import sys

sys.path.insert(0, "/opt/trn_rl_repo")

import numpy as np
import ml_dtypes

import concourse.bass as bass
import concourse.bacc as bacc
import concourse.mybir as mybir
import concourse.tile as tile
from concourse.bass_utils import run_bass_kernel_spmd
from concourse.masks import make_identity

BF16 = ml_dtypes.bfloat16

# ---------------- configuration (overridable for small-scale testing) ----
N = 50000          # nodes
H, D = 4, 64       # heads, head dim
NEG = 0.2          # leaky relu slope
C = 8              # cores
KC = 32            # max slot columns per chunk (SBUF bound)
LO_CAP = 32768     # rows in the low table region (int16 index bound)
PAD_EL = -30000.0  # el of the pad row => exp(score) == 0
ROW = 384          # bf16 elems/row: h[0:256] | el f32 @ 256:264 | er f32 @ 264:272 | pad
GRP = 8            # phase-A tiles per group
DEBUG = False      # add intermediate dump outputs

HD = H * D


def _derived(n):
    npc = n // C
    npad = -(-npc // 128) * 128
    nb = npad // 128
    npos = C * npad
    top = max(npos, n)
    hi = top - (LO_CAP - 1) + 1 if top >= LO_CAP - 1 else 1
    hi_cap = -(-(hi + 1) // 64) * 64
    return npc, npad, nb, npos, hi_cap


# ---------------------------------------------------------------- planning
def _pack_idx(flat):
    """dma_gather index stream: index i lives at [i % 16, i // 16]; the
    16-row group is replicated 8x across 128 partitions."""
    flat = np.asarray(flat, np.int16)
    a = flat.reshape(-1, 16).T
    return np.tile(a, (8, 1))


def _cumcount(key):
    n = len(key)
    starts = np.where(np.r_[True, key[1:] != key[:-1]], np.arange(n), 0)
    return np.arange(n) - np.maximum.accumulate(starts)


def build_plan(src, dst, n=None):
    n = n or N
    npc, npad, nb, npos, hi_cap = _derived(n)
    hi_pad = hi_cap - 1
    lo_pad = LO_CAP - 1
    src = np.asarray(src, np.int64)
    dst = np.asarray(dst, np.int64)
    ecore = dst // npc
    dloc = dst % npc

    layers = []
    percore = [dict() for _ in range(C)]
    rank1 = np.empty(n, np.int64)

    for layer in (0, 1):
        if layer == 0:
            spos = src
        else:
            pos2 = (np.arange(n) // npc) * npad + rank1
            spos = pos2[src]
        islo = (spos <= LO_CAP - 2).astype(np.int64)
        srclo = np.minimum(spos, lo_pad)
        srchi = np.maximum(spos - (LO_CAP - 1), 0)

        deg_lo = np.zeros((C, npc), np.int64)
        deg_hi = np.zeros((C, npc), np.int64)
        for c in range(C):
            m = ecore == c
            np.add.at(deg_lo[c], dloc[m][islo[m] == 1], 1)
            np.add.at(deg_hi[c], dloc[m][islo[m] == 0], 1)

        orders = []
        ranks = np.empty((C, npc), np.int64)
        KLc = np.zeros((C, nb), np.int64)
        KHc = np.zeros((C, nb), np.int64)
        for c in range(C):
            o = np.lexsort((-deg_hi[c], -deg_lo[c]))
            orders.append(o)
            ranks[c, o] = np.arange(npc)
            dl = np.zeros(npad, np.int64)
            dh = np.zeros(npad, np.int64)
            dl[:npc] = deg_lo[c][o]
            dh[:npc] = deg_hi[c][o]
            KLc[c] = dl.reshape(nb, 128).max(1)
            KHc[c] = dh.reshape(nb, 128).max(1)
        KL = KLc.max(0)
        KH = KHc.max(0)

        # shared chunk structure
        blocks = []
        for b in range(nb):
            chunks, cur, cur_w = [], [], 0
            for tab, w in ((0, int(KL[b])), (1, int(KH[b]))):
                while w > 0:
                    take = min(w, KC - cur_w)
                    cur.append([tab, take])
                    cur_w += take
                    w -= take
                    if cur_w == KC:
                        chunks.append(cur)
                        cur, cur_w = [], 0
            if cur:
                chunks.append(cur)
            if not chunks:
                chunks = [[[0, 1]]]
            blocks.append(chunks)

        for c in range(C):
            m = ecore == c
            dr = ranks[c][dloc[m]]
            lo_e = islo[m]
            # per-block slot grids
            grids = []
            for b in range(nb):
                w = int(KL[b] + KH[b])
                if w == 0:
                    w = 1
                g = np.empty((128, w), np.int16)
                g[:, : int(KL[b])] = lo_pad
                g[:, int(KL[b]):] = hi_pad
                if KL[b] + KH[b] == 0:
                    g[:] = lo_pad
                grids.append(g)
            if m.sum() > 0:
                order = np.lexsort((1 - lo_e, dr))
                r_s = dr[order]
                lo_s = lo_e[order]
                j = _cumcount(r_s * 2 + (1 - lo_s))
                srow = np.where(lo_s == 1, srclo[m][order], srchi[m][order])
                b_s = r_s // 128
                p_s = r_s % 128
                for b in range(nb):
                    mb = b_s == b
                    if not mb.any():
                        continue
                    col = np.where(lo_s[mb] == 1, j[mb], KL[b] + j[mb])
                    grids[b][p_s[mb], col] = srow[mb]
            parts = []
            for b in range(nb):
                g = grids[b]
                col0 = 0
                for ch in blocks[b]:
                    for tab, w in ch:
                        sub = g[:, col0 : col0 + w]
                        parts.append(_pack_idx(sub.T.ravel()))
                        col0 += w
            gi = np.concatenate(parts, axis=1)
            glob = np.full(npad, -1, np.int64)
            glob[:npc] = c * npc + orders[c]
            loc = np.maximum(glob - c * npc, 0)
            if layer == 0:
                p_of = glob.copy()
            else:
                p_of = np.where(glob >= 0, c * npad + ranks[c][loc], -1)
            trow = np.where(p_of < 0, lo_pad, p_of + (p_of >= LO_CAP - 1)).astype(np.int32)
            percore[c]["gidx%d" % (layer + 1)] = gi
            percore[c]["eidx%d" % (layer + 1)] = trow.reshape(nb, 128).T.copy()
            if layer == 1:
                percore[c]["order2"] = glob

        if layer == 0:
            for c in range(C):
                rank1[c * npc : (c + 1) * npc] = ranks[c]

        layers.append(dict(KL=KL, KH=KH, blocks=blocks))

    return dict(layers=layers, percore=percore, n=n, npc=npc, npad=npad,
                nb=nb, npos=npos, hi_cap=hi_cap)


# ---------------------------------------------------------- host weights
def _fold_weights(W, attn_l, attn_r):
    W = np.asarray(W, np.float32)
    Wh = W.reshape(64, H, D)
    wl = np.einsum("khd,hd->kh", Wh, np.asarray(attn_l, np.float32))
    wr = np.einsum("khd,hd->kh", Wh, np.asarray(attn_r, np.float32))
    return np.concatenate([W, wl, wr], axis=1)


def _pad_row():
    r = np.zeros(ROW // 2, np.float32)
    r[128:132] = PAD_EL
    return r.view(BF16)


# ------------------------------------------------------------- program
def build_program(plan):
    npad, nb, npos, hi_cap = plan["npad"], plan["nb"], plan["npos"], plan["hi_cap"]
    ntab = LO_CAP + hi_cap
    ntile = -(-npos // 128)
    assert ntile == C * nb and ntile % GRP == 0 and GRP % 2 == 0
    f32, bf16 = mybir.dt.float32, mybir.dt.bfloat16
    i16, i32 = mybir.dt.int16, mybir.dt.int32
    AX = mybir.AxisListType.X
    OP = mybir.AluOpType
    AF = mybir.ActivationFunctionType

    nc = bacc.Bacc("TRN2", target_bir_lowering=False, debug=False, num_devices=C)

    xTp = nc.dram_tensor("xTp", [128, ntile * 128], f32, kind="ExternalInput").ap()
    wcat1 = nc.dram_tensor("wcat1", [128, 320], f32, kind="ExternalInput").ap()
    wcat2 = nc.dram_tensor("wcat2", [128, 320], bf16, kind="ExternalInput").ap()
    bias1r = nc.dram_tensor("bias1r", [128, 256], f32, kind="ExternalInput").ap()
    bias2r = nc.dram_tensor("bias2r", [128, 256], f32, kind="ExternalInput").ap()
    padrow = nc.dram_tensor("padrow", [2, ROW], bf16, kind="ExternalInput").ap()
    X1 = plan["percore"][0]["gidx1"].shape[1]
    X2 = plan["percore"][0]["gidx2"].shape[1]
    gidx1 = nc.dram_tensor("gidx1", [128, X1], i16, kind="ExternalInput").ap()
    gidx2 = nc.dram_tensor("gidx2", [128, X2], i16, kind="ExternalInput").ap()
    eidx1 = nc.dram_tensor("eidx1", [128, nb], i32, kind="ExternalInput").ap()
    eidx2 = nc.dram_tensor("eidx2", [128, nb], i32, kind="ExternalInput").ap()
    xout = nc.dram_tensor("xout", [npad, 64], f32, kind="ExternalOutput").ap()

    if DEBUG:
        dbg_t1 = nc.dram_tensor("dbg_t1", [ntab, ROW], bf16, kind="ExternalOutput").ap()
        dbg_t2 = nc.dram_tensor("dbg_t2", [ntab, ROW], bf16, kind="ExternalOutput").ap()
        dbg_er1 = nc.dram_tensor("dbg_er1", [128, 0 + (plan["nb"] * 16)], bf16, kind="ExternalOutput").ap()
        dbg_x2T = nc.dram_tensor("dbg_x2T", [128, npad], bf16, kind="ExternalOutput").ap()
        dbg_ag = nc.dram_tensor("dbg_ag", [C * 128, npad], bf16, kind="ExternalOutput").ap()
    table1 = nc.dram_tensor("table1", [ntab, ROW], bf16).ap()
    table2 = nc.dram_tensor("table2", [ntab, ROW], bf16).ap()
    x2T = nc.dram_tensor("x2T", [128, npad], bf16).ap()
    ag = nc.dram_tensor("agx2", [C * 128, npad], bf16, addr_space="Shared").ap()

    bnd = LO_CAP - 1  # position that maps to the hi region

    with tile.TileContext(nc) as tc:
        with (
            tc.tile_pool(name="const", bufs=1) as constp,
            tc.tile_pool(name="sbuf", bufs=2) as sb,
            tc.tile_pool(name="psum", bufs=2, space="PSUM") as ps,
        ):
            ident = constp.tile([128, 128], bf16)
            make_identity(nc, ident[:])
            zer = constp.tile([64, 2048], bf16)
            nc.gpsimd.memset(zer[:], 0.0)
            for o in range(0, npad, 2048):
                w = min(2048, npad - o)
                nc.sync.dma_start(out=x2T[64:128, o : o + w], in_=zer[:, :w])

            b1t = constp.tile([128, 256], f32)
            nc.sync.dma_start(out=b1t[:], in_=bias1r[:])
            b2t = constp.tile([128, 256], f32)
            nc.sync.dma_start(out=b2t[:], in_=bias2r[:])
            w1t = constp.tile([128, 320], f32)
            nc.sync.dma_start(out=w1t[:], in_=wcat1[:])
            w2t = constp.tile([128, 320], bf16)
            nc.sync.dma_start(out=w2t[:], in_=wcat2[:])

            for layer in (0, 1):
                L = plan["layers"][layer]
                tab = table1 if layer == 0 else table2
                tab_lo = tab[0:LO_CAP]
                tab_hi = tab[LO_CAP:ntab]
                gidx = gidx1 if layer == 0 else gidx2
                eidx = eidx1 if layer == 0 else eidx2
                XX = X1 if layer == 0 else X2
                bias_t = b1t if layer == 0 else b2t
                wt = w1t if layer == 0 else w2t

                # ---------------- phase A: build the table --------------
                for g0 in range(0, ntile, GRP):
                    if layer == 0:
                        lh = sb.tile([128, GRP * 128], f32, tag="lhsA")
                        nc.sync.dma_start(out=lh[:],
                                          in_=xTp[:, g0 * 128 : (g0 + GRP) * 128])
                    else:
                        lh = sb.tile([128, GRP * 128], bf16, tag="lhsAb")
                        u = g0
                        while u < g0 + GRP:
                            c_ = u // nb
                            j0 = u % nb
                            run = min(g0 + GRP - u, nb - j0)
                            nc.sync.dma_start(
                                out=lh[:, (u - g0) * 128 : (u - g0 + run) * 128],
                                in_=ag[c_ * 128 : (c_ + 1) * 128,
                                       j0 * 128 : (j0 + run) * 128],
                            )
                            u += run
                    rows = sb.tile([128, GRP, ROW], bf16, tag="rowsA")
                    rowsap = rows[:]
                    rows32 = rowsap.bitcast(f32)
                    for pair in range(GRP // 2):
                        pt = ps.tile([128, 1024], f32, tag="psA")
                        for s in range(2):
                            u = pair * 2 + s
                            nc.tensor.matmul(
                                pt[:, s * 512 : s * 512 + 320],
                                lhsT=lh[:, u * 128 : (u + 1) * 128],
                                rhs=wt[:],
                                start=True, stop=True,
                            )
                        ptap = pt[:]
                        hsrc = bass.AP(ptap.tensor, ptap.offset,
                                       [ptap.ap[0], [512, 2], [1, 256]])
                        hdst = bass.AP(rowsap.tensor, rowsap.offset + pair * 2 * ROW,
                                       [rowsap.ap[0], [ROW, 2], [1, 256]])
                        esrc = bass.AP(ptap.tensor, ptap.offset + 256,
                                       [ptap.ap[0], [512, 2], [1, 64]])
                        edst = bass.AP(rows32.tensor,
                                       rows32.offset + pair * 2 * (ROW // 2) + 128,
                                       [rows32.ap[0], [ROW // 2, 2], [1, 64]])
                        if pair % 2 == 0:
                            nc.vector.tensor_copy(out=hdst, in_=hsrc)
                            nc.vector.tensor_copy(out=edst, in_=esrc)
                        else:
                            nc.scalar.copy(out=hdst, in_=hsrc)
                            nc.scalar.copy(out=edst, in_=esrc)
                    # store rows -> table, skipping over the lo/hi pad row
                    r0 = g0 * 128
                    nrows = GRP * 128
                    if r0 + nrows <= bnd:
                        segs = [(0, nrows, 0)]
                    elif r0 >= bnd:
                        segs = [(0, nrows, 1)]
                    else:
                        segs = [(0, bnd - r0, 0), (bnd - r0, nrows, 1)]
                    for a0, a1, shift in segs:
                        a = a0
                        while a < a1:
                            s0, p0 = divmod(a, 128)
                            dstr = r0 + a + shift
                            if p0 == 0 and a1 - a >= 128:
                                ns = (a1 - a) // 128
                                srcap = bass.AP(
                                    rowsap.tensor, rowsap.offset + s0 * ROW,
                                    [[rowsap.ap[0][0], 128], [ROW, ns], [1, ROW]])
                                dstap = bass.AP(
                                    tab.tensor, tab.offset + dstr * ROW,
                                    [[ROW, 128], [128 * ROW, ns], [1, ROW]])
                                nc.sync.dma_start(out=dstap, in_=srcap)
                                a += ns * 128
                            else:
                                cnt = min(a1 - a, 128 - p0)
                                srcap = bass.AP(
                                    rowsap.tensor,
                                    rowsap.offset + s0 * ROW,
                                    [[rowsap.ap[0][0], 128], [1, ROW]])[p0 : p0 + cnt]
                                nc.sync.dma_start(
                                    out=tab[dstr : dstr + cnt], in_=srcap)
                                a += cnt
                nc.sync.dma_start(out=tab[bnd : bnd + 1], in_=padrow[0:1])
                nc.sync.dma_start(out=tab[ntab - 1 : ntab], in_=padrow[1:2])

                tc.strict_bb_all_engine_barrier()
                if DEBUG:
                    nc.sync.dma_start(out=(dbg_t1 if layer == 0 else dbg_t2), in_=tab)

                # ---------------- edge phase ----------------------------
                idx_sb = constp.tile([128, XX], i16, tag="gidx%d" % layer)
                nc.sync.dma_start(out=idx_sb[:], in_=gidx[:])
                eidx_sb = constp.tile([128, nb], i32, tag="eidx%d" % layer)
                nc.sync.dma_start(out=eidx_sb[:], in_=eidx[:])
                er_sb = sb.tile([128, nb * 16], bf16, tag="er")
                er32 = er_sb[:].bitcast(f32)
                for b in range(nb):
                    nc.gpsimd.indirect_dma_start(
                        out=er_sb[:, b * 16 : (b + 1) * 16],
                        out_offset=None,
                        in_=tab,
                        in_offset=bass.IndirectOffsetOnAxis(
                            ap=eidx_sb[:, b : b + 1], axis=0),
                        element_offset=256,
                    )

                if DEBUG and layer == 0:
                    nc.sync.dma_start(out=dbg_er1, in_=er_sb[:])
                x2stage = (sb.tile([64, nb * 128], bf16, tag="x2s", name="x2stage")
                           if layer == 0 else None)
                icol = 0
                for b in range(nb):
                    chunks = L["blocks"][b]
                    num = None     # (tensor, offset, pstride) of a [128,256] value
                    zacc = None
                    for ci, ch in enumerate(chunks):
                        w = sum(x[1] for x in ch)
                        G = sb.tile([128, KC, ROW], bf16, tag="G")
                        c0 = 0
                        for tabi, cw in ch:
                            nidx = 128 * cw
                            nc.gpsimd.dma_gather(
                                G[:, c0 : c0 + cw, :],
                                tab_lo if tabi == 0 else tab_hi,
                                idx_sb[:, icol : icol + nidx // 16],
                                nidx, nidx, ROW,
                                single_packet=False,
                            )
                            icol += nidx // 16
                            c0 += cw
                        Ga = G[:]
                        elf = Ga[:, :w, 256:272].bitcast(f32)[:, :, 0:4]
                        erb = bass.AP(er32.tensor, er32.offset + b * 8 + 4,
                                      [er32.ap[0], [0, w], [1, 4]])
                        S = sb.tile([128, KC * 4], f32, tag="S")
                        S3 = S[:, : w * 4].rearrange("p (k f) -> p k f", f=4)
                        nc.vector.tensor_tensor(out=S3, in0=elf, in1=erb, op=OP.add)
                        T = sb.tile([128, KC * 4], f32, tag="T")
                        nc.vector.tensor_scalar_mul(
                            out=T[:, : w * 4], in0=S[:, : w * 4], scalar1=NEG)
                        nc.vector.tensor_tensor(
                            out=S[:, : w * 4], in0=S[:, : w * 4],
                            in1=T[:, : w * 4], op=OP.max)
                        P4 = sb.tile([128, KC * 4], f32, tag="P4")
                        nc.scalar.activation(
                            out=P4[:, : w * 4], in_=S[:, : w * 4], func=AF.Exp)
                        zc = sb.tile([128, 4], f32, tag="zc")
                        p4ap = P4[:]
                        p4perm = bass.AP(p4ap.tensor, p4ap.offset,
                                         [p4ap.ap[0], [1, 4], [4, w]])
                        nc.vector.tensor_reduce(out=zc[:], in_=p4perm, axis=AX, op=OP.add)
                        PE = sb.tile([128, KC, 256], bf16, tag="PE")
                        sap = S[:]
                        sbc = bass.AP(sap.tensor, sap.offset,
                                      [sap.ap[0], [4, w], [1, 4], [0, 64]])
                        pe4 = PE[:, :w, :].rearrange("p k (h d) -> p k h d", d=64)
                        nc.scalar.activation(out=pe4, in_=sbc, func=AF.Exp)
                        nc.vector.tensor_tensor(
                            out=PE[:, :w, :], in0=Ga[:, :w, 0:256],
                            in1=PE[:, :w, :], op=OP.mult)
                        k = w
                        while k > 1:
                            hh = k // 2
                            ce = k - hh
                            nc.vector.tensor_tensor(
                                out=PE[:, :hh, :], in0=PE[:, :hh, :],
                                in1=PE[:, ce : ce + hh, :], op=OP.add)
                            k = ce
                        peap = PE[:]
                        if ci == 0:
                            num = (peap.tensor, peap.offset, peap.ap[0])
                            zacc = zc
                        else:
                            nu = sb.tile([128, 256], f32, tag="nacc")
                            prev = bass.AP(num[0], num[1], [num[2], [1, 256]])
                            cur = bass.AP(peap.tensor, peap.offset, [peap.ap[0], [1, 256]])
                            nc.vector.tensor_tensor(out=nu[:], in0=prev, in1=cur, op=OP.add)
                            nuap = nu[:]
                            num = (nuap.tensor, nuap.offset, nuap.ap[0])
                            nc.vector.tensor_tensor(out=zacc[:], in0=zacc[:], in1=zc[:], op=OP.add)
                    # ---- node epilogue
                    nc.vector.tensor_scalar_max(out=zacc[:], in0=zacc[:], scalar1=1e-30)
                    rz = sb.tile([128, 4], f32, tag="rz")
                    nc.vector.reciprocal(rz[:], zacc[:])
                    rzap = rz[:]
                    rzb = bass.AP(rzap.tensor, rzap.offset,
                                  [rzap.ap[0], [1, 4], [0, 64]])
                    numv = bass.AP(num[0], num[1], [num[2], [64, 4], [1, 64]])
                    O = sb.tile([128, 256], f32, tag="O")
                    nc.vector.tensor_tensor(
                        out=O[:].rearrange("p (h d) -> p h d", d=64),
                        in0=numv, in1=rzb, op=OP.mult)
                    nc.vector.tensor_tensor(out=O[:], in0=O[:], in1=bias_t[:], op=OP.add)
                    if layer == 0:
                        O2 = sb.tile([128, 256], f32, tag="O2")
                        nc.scalar.activation(out=O2[:], in_=O[:], func=AF.Tanh)
                    else:
                        O2 = O
                    m1 = sb.tile([128, 64], f32, tag="m1")
                    m2 = sb.tile([128, 64], f32, tag="m2")
                    nc.vector.tensor_tensor(out=m1[:], in0=O2[:, 0:64],
                                            in1=O2[:, 64:128], op=OP.add)
                    nc.vector.tensor_tensor(out=m2[:], in0=O2[:, 128:192],
                                            in1=O2[:, 192:256], op=OP.add)
                    nc.vector.tensor_tensor(out=m1[:], in0=m1[:], in1=m2[:], op=OP.add)
                    if layer == 0:
                        xb = sb.tile([128, 64], bf16, tag="xb")
                        nc.vector.tensor_scalar_mul(out=xb[:], in0=m1[:], scalar1=0.25)
                        pt = ps.tile([64, 128], bf16, tag="psT")
                        nc.tensor.transpose(out=pt[:], in_=xb[:], identity=ident[:])
                        nc.vector.tensor_copy(
                            out=x2stage[:, b * 128 : (b + 1) * 128], in_=pt[:])
                    else:
                        xo = sb.tile([128, 64], f32, tag="xo")
                        nc.vector.tensor_scalar_mul(out=xo[:], in0=m1[:], scalar1=0.25)
                        nc.sync.dma_start(out=xout[b * 128 : (b + 1) * 128, :], in_=xo[:])

                if layer == 0:
                    nc.sync.dma_start(out=x2T[0:64, :], in_=x2stage[:])
                    tc.strict_bb_all_engine_barrier()
                    nc.gpsimd.collective_compute(
                        "AllGather", OP.bypass,
                        replica_groups=[list(range(C))],
                        ins=[x2T], outs=[ag],
                    )
                    tc.strict_bb_all_engine_barrier()
                    if DEBUG:
                        nc.sync.dma_start(out=dbg_x2T, in_=x2T)
                        nc.sync.dma_start(out=dbg_ag, in_=ag)

    nc.compile()
    return nc


# ------------------------------------------------------------- entry point
_CACHE = {}


def get_compiled(src, dst, n):
    key = (n, hash(np.asarray(src).tobytes()) ^ hash(np.asarray(dst).tobytes()))
    if key not in _CACHE:
        plan = build_plan(src, dst, n)
        nc = build_program(plan)
        _CACHE[key] = (plan, nc)
    return _CACHE[key]


def make_inputs(plan, x, W1, attn_l1, attn_r1, bias1, W2, attn_l2, attn_r2, bias2):
    n, npos = plan["n"], plan["npos"]
    ntile = -(-npos // 128)
    xTp = np.zeros((128, ntile * 128), np.float32)
    xTp[:64, :n] = np.asarray(x, np.float32).T
    wc1 = np.zeros((128, 320), np.float32)
    wc1[:64, :264] = _fold_weights(W1, attn_l1, attn_r1)
    wc2 = np.zeros((128, 320), np.float32)
    wc2[:64, :264] = _fold_weights(W2, attn_l2, attn_r2)
    pr = _pad_row()
    common = dict(
        xTp=xTp,
        wcat1=wc1,
        wcat2=wc2.astype(BF16),
        bias1r=np.tile(np.asarray(bias1, np.float32)[None, :], (128, 1)),
        bias2r=np.tile(np.asarray(bias2, np.float32)[None, :], (128, 1)),
        padrow=np.stack([pr, pr]),
    )
    in_maps = []
    for c in range(C):
        pc = plan["percore"][c]
        m = dict(common)
        for k in ("gidx1", "gidx2", "eidx1", "eidx2"):
            m[k] = pc[k]
        in_maps.append(m)
    return in_maps


def kernel(x, src, dst, W1, attn_l1, attn_r1, bias1, W2, attn_l2, attn_r2, bias2):
    x = np.asarray(x)
    n = x.shape[0]
    src = np.asarray(src, np.int64)
    dst = np.asarray(dst, np.int64)
    plan, nc = get_compiled(src, dst, n)
    in_maps = make_inputs(plan, x, W1, attn_l1, attn_r1, bias1,
                          W2, attn_l2, attn_r2, bias2)
    res = run_bass_kernel_spmd(nc, in_maps, list(range(C)))
    out = np.empty((n, 64), np.float32)
    for c in range(C):
        xo = res.results[c]["xout"]
        o2 = plan["percore"][c]["order2"]
        real = o2 >= 0
        out[o2[real]] = xo[real]
    return out


# revision 10
# speedup vs baseline: 8.2476x; 8.2476x over previous
"""Trainium2 Bass kernel for a 2-layer GAT (nn_GAT_Module_90623809945643).

Strategy (8 NeuronCores, SPMD):
  - Nodes are partitioned across the 8 cores by global id (6250 each); each
    core owns the edges whose dst it owns, so edge-softmax and segment-sum
    stay local (dst-owner bucketing).
  - Per layer, every core builds the full per-node feature table
    [h (bf16, 256) | el (f32, 4) | er (f32, 4) | pad] (768B rows) from
    replicated x (layer 1) / the all-gathered layer-1 output (layer 2) with
    TensorE matmuls; attn_l/attn_r are folded into the weight matrix on the
    host (el = x @ (W_h @ attn_l_h)), so el/er are matmul outputs.
  - Edge phase: nodes-on-partitions layout. The host sorts each core's nodes
    by in-degree and packs each node's incoming edges into fixed-width slot
    grids (degree bucketing); h[src]/el[src] rows are fetched with dma_gather
    (int16 indices, table split at the 32768-row boundary), er comes
    per-partition via indirect DMA. The weighted segment-sum runs as
    broadcast-exp (ACT) + multiply + pairwise-tree reduction (DVE); the
    softmax division happens once per node after aggregation (the reference's
    max-subtraction cancels exactly and is skipped; scores are O(1) so exp
    cannot overflow).
  - One AllGather (bf16, transposed layer-1 output) between the layers.

The host performs index-only preprocessing (bucketing, padding, index
streams); all floating-point work on x happens on the device.
"""

import sys

sys.path.insert(0, "/opt/trn_rl_repo")

import numpy as np
import ml_dtypes

import concourse.bass as bass
import concourse.bacc as bacc
import concourse.mybir as mybir
import concourse.tile as tile
from concourse.bass_utils import run_bass_kernel_spmd
from concourse.masks import make_identity

BF16 = ml_dtypes.bfloat16

# ---------------- configuration (overridable for small-scale testing) ----
N = 50000          # nodes
H, D = 4, 64       # heads, head dim
NEG = 0.2          # leaky relu slope
C = 8              # cores
KC = 32            # max slot columns per chunk (SBUF bound)
LO_CAP = 32768     # rows in the low table region (int16 index bound)
PAD_EL = -30000.0  # el of the pad row => exp(score) == 0
ROW = 384          # bf16 elems/row: h[0:256] | el f32 @ 256:264 | er f32 @ 264:272 | pad
GRP = 8            # phase-A tiles per group
DEBUG = False      # add intermediate dump outputs
PROFILE_SINGLE = False  # replace collective with local DMAs (TimelineSim profiling)
REPEAT = 1         # repeat the whole 2-layer computation (timing slope)

HD = H * D


def _derived(n):
    npc = n // C
    npad = -(-npc // 128) * 128
    nb = npad // 128
    npos = C * npad
    top = max(npos, n)
    hi = top - (LO_CAP - 1) + 1 if top >= LO_CAP - 1 else 1
    hi_cap = -(-(hi + 1) // 64) * 64
    return npc, npad, nb, npos, hi_cap


# ---------------------------------------------------------------- planning
def _pack_idx(flat):
    """dma_gather index stream: index i lives at [i % 16, i // 16]; the
    16-row group is replicated 8x across 128 partitions."""
    flat = np.asarray(flat, np.int16)
    a = flat.reshape(-1, 16).T
    return np.tile(a, (8, 1))


def _cumcount(key):
    n = len(key)
    starts = np.where(np.r_[True, key[1:] != key[:-1]], np.arange(n), 0)
    return np.arange(n) - np.maximum.accumulate(starts)


def build_plan(src, dst, n=None):
    n = n or N
    npc, npad, nb, npos, hi_cap = _derived(n)
    hi_pad = hi_cap - 1
    lo_pad = LO_CAP - 1
    src = np.asarray(src, np.int64)
    dst = np.asarray(dst, np.int64)
    ecore = dst // npc
    dloc = dst % npc

    layers = []
    percore = [dict() for _ in range(C)]
    rank1 = np.empty(n, np.int64)

    for layer in (0, 1):
        if layer == 0:
            spos = src
        else:
            pos2 = (np.arange(n) // npc) * npad + rank1
            spos = pos2[src]
        islo = (spos <= LO_CAP - 2).astype(np.int64)
        srclo = np.minimum(spos, lo_pad)
        srchi = np.maximum(spos - (LO_CAP - 1), 0)

        deg_lo = np.zeros((C, npc), np.int64)
        deg_hi = np.zeros((C, npc), np.int64)
        for c in range(C):
            m = ecore == c
            np.add.at(deg_lo[c], dloc[m][islo[m] == 1], 1)
            np.add.at(deg_hi[c], dloc[m][islo[m] == 0], 1)

        orders = []
        ranks = np.empty((C, npc), np.int64)
        KLc = np.zeros((C, nb), np.int64)
        KHc = np.zeros((C, nb), np.int64)
        for c in range(C):
            o = np.lexsort((-deg_hi[c], -deg_lo[c]))
            orders.append(o)
            ranks[c, o] = np.arange(npc)
            dl = np.zeros(npad, np.int64)
            dh = np.zeros(npad, np.int64)
            dl[:npc] = deg_lo[c][o]
            dh[:npc] = deg_hi[c][o]
            KLc[c] = dl.reshape(nb, 128).max(1)
            KHc[c] = dh.reshape(nb, 128).max(1)
        KL = KLc.max(0)
        KH = KHc.max(0)

        # shared chunk structure
        blocks = []
        for b in range(nb):
            chunks, cur, cur_w = [], [], 0
            for tab, w in ((0, int(KL[b])), (1, int(KH[b]))):
                while w > 0:
                    take = min(w, KC - cur_w)
                    cur.append([tab, take])
                    cur_w += take
                    w -= take
                    if cur_w == KC:
                        chunks.append(cur)
                        cur, cur_w = [], 0
            if cur:
                chunks.append(cur)
            if not chunks:
                chunks = [[[0, 1]]]
            blocks.append(chunks)

        for c in range(C):
            m = ecore == c
            dr = ranks[c][dloc[m]]
            lo_e = islo[m]
            # per-block slot grids
            grids = []
            for b in range(nb):
                w = int(KL[b] + KH[b])
                if w == 0:
                    w = 1
                g = np.empty((128, w), np.int16)
                g[:, : int(KL[b])] = lo_pad
                g[:, int(KL[b]):] = hi_pad
                if KL[b] + KH[b] == 0:
                    g[:] = lo_pad
                grids.append(g)
            if m.sum() > 0:
                order = np.lexsort((1 - lo_e, dr))
                r_s = dr[order]
                lo_s = lo_e[order]
                j = _cumcount(r_s * 2 + (1 - lo_s))
                srow = np.where(lo_s == 1, srclo[m][order], srchi[m][order])
                b_s = r_s // 128
                p_s = r_s % 128
                for b in range(nb):
                    mb = b_s == b
                    if not mb.any():
                        continue
                    col = np.where(lo_s[mb] == 1, j[mb], KL[b] + j[mb])
                    grids[b][p_s[mb], col] = srow[mb]
            parts = []
            for b in range(nb):
                g = grids[b]
                col0 = 0
                for ch in blocks[b]:
                    for tab, w in ch:
                        sub = g[:, col0 : col0 + w]
                        parts.append(_pack_idx(sub.T.ravel()))
                        col0 += w
            gi = np.concatenate(parts, axis=1)
            glob = np.full(npad, -1, np.int64)
            glob[:npc] = c * npc + orders[c]
            loc = np.maximum(glob - c * npc, 0)
            if layer == 0:
                p_of = glob.copy()
            else:
                p_of = np.where(glob >= 0, c * npad + ranks[c][loc], -1)
            trow = np.where(p_of < 0, lo_pad, p_of + (p_of >= LO_CAP - 1)).astype(np.int32)
            percore[c]["gidx%d" % (layer + 1)] = gi
            percore[c]["eidx%d" % (layer + 1)] = trow.reshape(nb, 128).T.copy()
            if layer == 1:
                percore[c]["order2"] = glob

        if layer == 0:
            for c in range(C):
                rank1[c * npc : (c + 1) * npc] = ranks[c]

        layers.append(dict(KL=KL, KH=KH, blocks=blocks))

    return dict(layers=layers, percore=percore, n=n, npc=npc, npad=npad,
                nb=nb, npos=npos, hi_cap=hi_cap)


# ---------------------------------------------------------- host weights
def _fold_weights(W, attn_l, attn_r):
    W = np.asarray(W, np.float32)
    Wh = W.reshape(64, H, D)
    wl = np.einsum("khd,hd->kh", Wh, np.asarray(attn_l, np.float32))
    wr = np.einsum("khd,hd->kh", Wh, np.asarray(attn_r, np.float32))
    return np.concatenate([W, wl, wr], axis=1)


def _pad_row():
    r = np.zeros(ROW // 2, np.float32)
    r[128:132] = PAD_EL
    return r.view(BF16)


# ------------------------------------------------------------- program
def build_program(plan):
    npad, nb, npos, hi_cap = plan["npad"], plan["nb"], plan["npos"], plan["hi_cap"]
    ntab = LO_CAP + hi_cap
    ntile = -(-npos // 128)
    assert ntile == C * nb and ntile % GRP == 0 and GRP % 2 == 0
    f32, bf16 = mybir.dt.float32, mybir.dt.bfloat16
    i16, i32 = mybir.dt.int16, mybir.dt.int32
    AX = mybir.AxisListType.X
    OP = mybir.AluOpType
    AF = mybir.ActivationFunctionType

    nc = bacc.Bacc("TRN2", target_bir_lowering=False, debug=False, num_devices=C,
                   num_swdge_queues=4)

    xTp = nc.dram_tensor("xTp", [128, ntile * 128], f32, kind="ExternalInput").ap()
    wcat1 = nc.dram_tensor("wcat1", [128, 320], f32, kind="ExternalInput").ap()
    wcat2 = nc.dram_tensor("wcat2", [128, 320], bf16, kind="ExternalInput").ap()
    bias1r = nc.dram_tensor("bias1r", [128, 256], f32, kind="ExternalInput").ap()
    bias2r = nc.dram_tensor("bias2r", [128, 256], f32, kind="ExternalInput").ap()
    padrow = nc.dram_tensor("padrow", [2, ROW], bf16, kind="ExternalInput").ap()
    X1 = plan["percore"][0]["gidx1"].shape[1]
    X2 = plan["percore"][0]["gidx2"].shape[1]
    gidx1 = nc.dram_tensor("gidx1", [128, X1], i16, kind="ExternalInput").ap()
    gidx2 = nc.dram_tensor("gidx2", [128, X2], i16, kind="ExternalInput").ap()
    eidx1 = nc.dram_tensor("eidx1", [128, nb], i32, kind="ExternalInput").ap()
    eidx2 = nc.dram_tensor("eidx2", [128, nb], i32, kind="ExternalInput").ap()
    xout = nc.dram_tensor("xout", [npad, 64], f32, kind="ExternalOutput").ap()

    if DEBUG:
        dbg_t1 = nc.dram_tensor("dbg_t1", [ntab, ROW], bf16, kind="ExternalOutput").ap()
        dbg_t2 = nc.dram_tensor("dbg_t2", [ntab, ROW], bf16, kind="ExternalOutput").ap()
        dbg_er1 = nc.dram_tensor("dbg_er1", [128, 0 + (plan["nb"] * 16)], bf16, kind="ExternalOutput").ap()
        dbg_x2T = nc.dram_tensor("dbg_x2T", [128, npad], bf16, kind="ExternalOutput").ap()
        dbg_ag = nc.dram_tensor("dbg_ag", [C * 128, npad], bf16, kind="ExternalOutput").ap()
    table1 = nc.dram_tensor("table1", [ntab, ROW], bf16).ap()
    table2 = nc.dram_tensor("table2", [ntab, ROW], bf16).ap()
    x2T = nc.dram_tensor("x2T", [128, npad], bf16).ap()
    ag = nc.dram_tensor("agx2", [C * 128, npad], bf16, addr_space="Shared").ap()

    bnd = LO_CAP - 1  # position that maps to the hi region

    with tile.TileContext(nc) as tc:
        with (
            tc.tile_pool(name="const", bufs=1) as constp,
            tc.tile_pool(name="sbuf", bufs=2) as sb,
            tc.tile_pool(name="psum", bufs=2, space="PSUM") as ps,
        ):
            ident = constp.tile([128, 128], bf16)
            make_identity(nc, ident[:])
            zer = constp.tile([64, 2048], bf16)
            nc.gpsimd.memset(zer[:], 0.0)
            for o in range(0, npad, 2048):
                w = min(2048, npad - o)
                nc.sync.dma_start(out=x2T[64:128, o : o + w], in_=zer[:, :w])

            b1t = constp.tile([128, 256], f32)
            nc.sync.dma_start(out=b1t[:], in_=bias1r[:])
            b2t = constp.tile([128, 256], f32)
            nc.sync.dma_start(out=b2t[:], in_=bias2r[:])
            w1t = constp.tile([128, 320], f32)
            nc.sync.dma_start(out=w1t[:], in_=wcat1[:])
            w2t = constp.tile([128, 320], bf16)
            nc.sync.dma_start(out=w2t[:], in_=wcat2[:])

            for _rep in range(REPEAT):
              for layer in (0, 1):
                L = plan["layers"][layer]
                tab = table1 if layer == 0 else table2
                tab_lo = tab[0:LO_CAP]
                tab_hi = tab[LO_CAP:ntab]
                gidx = gidx1 if layer == 0 else gidx2
                eidx = eidx1 if layer == 0 else eidx2
                XX = X1 if layer == 0 else X2
                bias_t = b1t if layer == 0 else b2t
                wt = w1t if layer == 0 else w2t

                # ---------------- phase A: build the table --------------
                for g0 in range(0, ntile, GRP):
                    if layer == 0:
                        lh = sb.tile([128, GRP * 128], f32, tag="lhsA")
                        nc.sync.dma_start(out=lh[:],
                                          in_=xTp[:, g0 * 128 : (g0 + GRP) * 128])
                    else:
                        lh = sb.tile([128, GRP * 128], bf16, tag="lhsAb")
                        u = g0
                        while u < g0 + GRP:
                            c_ = u // nb
                            j0 = u % nb
                            run = min(g0 + GRP - u, nb - j0)
                            nc.sync.dma_start(
                                out=lh[:, (u - g0) * 128 : (u - g0 + run) * 128],
                                in_=ag[c_ * 128 : (c_ + 1) * 128,
                                       j0 * 128 : (j0 + run) * 128],
                            )
                            u += run
                    rows = sb.tile([128, GRP, ROW], bf16, tag="rowsA")
                    rowsap = rows[:]
                    rows32 = rowsap.bitcast(f32)
                    for pair in range(GRP // 2):
                        pt = ps.tile([128, 1024], f32, tag="psA")
                        for s in range(2):
                            u = pair * 2 + s
                            nc.tensor.matmul(
                                pt[:, s * 512 : s * 512 + 320],
                                lhsT=lh[:, u * 128 : (u + 1) * 128],
                                rhs=wt[:],
                                start=True, stop=True,
                            )
                        ptap = pt[:]
                        hsrc = bass.AP(ptap.tensor, ptap.offset,
                                       [ptap.ap[0], [512, 2], [1, 256]])
                        hdst = bass.AP(rowsap.tensor, rowsap.offset + pair * 2 * ROW,
                                       [rowsap.ap[0], [ROW, 2], [1, 256]])
                        esrc = bass.AP(ptap.tensor, ptap.offset + 256,
                                       [ptap.ap[0], [512, 2], [1, 64]])
                        edst = bass.AP(rows32.tensor,
                                       rows32.offset + pair * 2 * (ROW // 2) + 128,
                                       [rows32.ap[0], [ROW // 2, 2], [1, 64]])
                        if pair % 2 == 0:
                            nc.vector.tensor_copy(out=hdst, in_=hsrc)
                            nc.vector.tensor_copy(out=edst, in_=esrc)
                        else:
                            nc.scalar.copy(out=hdst, in_=hsrc)
                            nc.scalar.copy(out=edst, in_=esrc)
                    # store rows -> table, skipping over the lo/hi pad row
                    r0 = g0 * 128
                    nrows = GRP * 128
                    if r0 + nrows <= bnd:
                        segs = [(0, nrows, 0)]
                    elif r0 >= bnd:
                        segs = [(0, nrows, 1)]
                    else:
                        segs = [(0, bnd - r0, 0), (bnd - r0, nrows, 1)]
                    for a0, a1, shift in segs:
                        a = a0
                        while a < a1:
                            s0, p0 = divmod(a, 128)
                            dstr = r0 + a + shift
                            if p0 == 0 and a1 - a >= 128:
                                ns = (a1 - a) // 128
                                srcap = bass.AP(
                                    rowsap.tensor, rowsap.offset + s0 * ROW,
                                    [[rowsap.ap[0][0], 128], [ROW, ns], [1, ROW]])
                                dstap = bass.AP(
                                    tab.tensor, tab.offset + dstr * ROW,
                                    [[ROW, 128], [128 * ROW, ns], [1, ROW]])
                                nc.sync.dma_start(out=dstap, in_=srcap)
                                a += ns * 128
                            else:
                                cnt = min(a1 - a, 128 - p0)
                                srcap = bass.AP(
                                    rowsap.tensor,
                                    rowsap.offset + s0 * ROW,
                                    [[rowsap.ap[0][0], 128], [1, ROW]])[p0 : p0 + cnt]
                                nc.sync.dma_start(
                                    out=tab[dstr : dstr + cnt], in_=srcap)
                                a += cnt
                nc.sync.dma_start(out=tab[bnd : bnd + 1], in_=padrow[0:1])
                nc.sync.dma_start(out=tab[ntab - 1 : ntab], in_=padrow[1:2])

                tc.strict_bb_all_engine_barrier()
                if DEBUG:
                    nc.sync.dma_start(out=(dbg_t1 if layer == 0 else dbg_t2), in_=tab)

                # ---------------- edge phase ----------------------------
                idx_sb = constp.tile([128, XX], i16, tag="gidx%d" % layer)
                nc.sync.dma_start(out=idx_sb[:], in_=gidx[:])
                eidx_sb = constp.tile([128, nb], i32, tag="eidx%d" % layer)
                nc.sync.dma_start(out=eidx_sb[:], in_=eidx[:])
                er_sb = sb.tile([128, nb * 16], bf16, tag="er")
                er32 = er_sb[:].bitcast(f32)
                for b in range(nb):
                    nc.gpsimd.indirect_dma_start(
                        out=er_sb[:, b * 16 : (b + 1) * 16],
                        out_offset=None,
                        in_=tab,
                        in_offset=bass.IndirectOffsetOnAxis(
                            ap=eidx_sb[:, b : b + 1], axis=0),
                        element_offset=256,
                    )

                if DEBUG and layer == 0:
                    nc.sync.dma_start(out=dbg_er1, in_=er_sb[:])
                x2stage = (sb.tile([64, nb * 128], bf16, tag="x2s", name="x2stage")
                           if layer == 0 else None)
                icol = 0
                gq = 0
                for b in range(nb):
                    chunks = L["blocks"][b]
                    num = None     # (tensor, offset, pstride) of a [128,256] value
                    zacc = None
                    for ci, ch in enumerate(chunks):
                        w = sum(x[1] for x in ch)
                        G = sb.tile([128, KC, ROW], bf16, tag="G")
                        c0 = 0
                        for tabi, cw in ch:
                            nidx = 128 * cw
                            nc.gpsimd.dma_gather(
                                G[:, c0 : c0 + cw, :],
                                tab_lo if tabi == 0 else tab_hi,
                                idx_sb[:, icol : icol + nidx // 16],
                                nidx, nidx, ROW,
                                single_packet=False, queue_num=gq % 4,
                            )
                            gq += 1
                            icol += nidx // 16
                            c0 += cw
                        Ga = G[:]
                        elf = Ga[:, :w, 256:272].bitcast(f32)[:, :, 0:4]
                        erb = bass.AP(er32.tensor, er32.offset + b * 8 + 4,
                                      [er32.ap[0], [0, w], [1, 4]])
                        S = sb.tile([128, KC * 4], f32, tag="S")
                        S3 = S[:, : w * 4].rearrange("p (k f) -> p k f", f=4)
                        nc.vector.tensor_tensor(out=S3, in0=elf, in1=erb, op=OP.add)
                        T = sb.tile([128, KC * 4], f32, tag="T")
                        nc.vector.tensor_scalar_mul(
                            out=T[:, : w * 4], in0=S[:, : w * 4], scalar1=NEG)
                        nc.vector.tensor_tensor(
                            out=S[:, : w * 4], in0=S[:, : w * 4],
                            in1=T[:, : w * 4], op=OP.max)
                        P4 = sb.tile([128, KC * 4], f32, tag="P4")
                        nc.scalar.activation(
                            out=P4[:, : w * 4], in_=S[:, : w * 4], func=AF.Exp)
                        zc = sb.tile([128, 4], f32, tag="zc")
                        p4ap = P4[:]
                        p4perm = bass.AP(p4ap.tensor, p4ap.offset,
                                         [p4ap.ap[0], [1, 4], [4, w]])
                        nc.vector.tensor_reduce(out=zc[:], in_=p4perm, axis=AX, op=OP.add)
                        PE = sb.tile([128, KC, 256], bf16, tag="PE")
                        sap = S[:]
                        sbc = bass.AP(sap.tensor, sap.offset,
                                      [sap.ap[0], [4, w], [1, 4], [0, 64]])
                        pe4 = PE[:, :w, :].rearrange("p k (h d) -> p k h d", d=64)
                        nc.scalar.activation(out=pe4, in_=sbc, func=AF.Exp)
                        nc.vector.tensor_tensor(
                            out=PE[:, :w, :], in0=Ga[:, :w, 0:256],
                            in1=PE[:, :w, :], op=OP.mult)
                        k = w
                        while k > 1:
                            hh = k // 2
                            ce = k - hh
                            nc.vector.tensor_tensor(
                                out=PE[:, :hh, :], in0=PE[:, :hh, :],
                                in1=PE[:, ce : ce + hh, :], op=OP.add)
                            k = ce
                        peap = PE[:]
                        if ci == 0:
                            num = (peap.tensor, peap.offset, peap.ap[0])
                            zacc = zc
                        else:
                            nu = sb.tile([128, 256], f32, tag="nacc")
                            prev = bass.AP(num[0], num[1], [num[2], [1, 256]])
                            cur = bass.AP(peap.tensor, peap.offset, [peap.ap[0], [1, 256]])
                            nc.vector.tensor_tensor(out=nu[:], in0=prev, in1=cur, op=OP.add)
                            nuap = nu[:]
                            num = (nuap.tensor, nuap.offset, nuap.ap[0])
                            nc.vector.tensor_tensor(out=zacc[:], in0=zacc[:], in1=zc[:], op=OP.add)
                    # ---- node epilogue
                    nc.vector.tensor_scalar_max(out=zacc[:], in0=zacc[:], scalar1=1e-30)
                    rz = sb.tile([128, 4], f32, tag="rz")
                    nc.vector.reciprocal(rz[:], zacc[:])
                    rzap = rz[:]
                    rzb = bass.AP(rzap.tensor, rzap.offset,
                                  [rzap.ap[0], [1, 4], [0, 64]])
                    numv = bass.AP(num[0], num[1], [num[2], [64, 4], [1, 64]])
                    O = sb.tile([128, 256], f32, tag="O")
                    nc.vector.tensor_tensor(
                        out=O[:].rearrange("p (h d) -> p h d", d=64),
                        in0=numv, in1=rzb, op=OP.mult)
                    nc.vector.tensor_tensor(out=O[:], in0=O[:], in1=bias_t[:], op=OP.add)
                    if layer == 0:
                        O2 = sb.tile([128, 256], f32, tag="O2")
                        nc.scalar.activation(out=O2[:], in_=O[:], func=AF.Tanh)
                    else:
                        O2 = O
                    m1 = sb.tile([128, 64], f32, tag="m1")
                    m2 = sb.tile([128, 64], f32, tag="m2")
                    nc.vector.tensor_tensor(out=m1[:], in0=O2[:, 0:64],
                                            in1=O2[:, 64:128], op=OP.add)
                    nc.vector.tensor_tensor(out=m2[:], in0=O2[:, 128:192],
                                            in1=O2[:, 192:256], op=OP.add)
                    nc.vector.tensor_tensor(out=m1[:], in0=m1[:], in1=m2[:], op=OP.add)
                    if layer == 0:
                        xb = sb.tile([128, 64], bf16, tag="xb")
                        nc.vector.tensor_scalar_mul(out=xb[:], in0=m1[:], scalar1=0.25)
                        pt = ps.tile([64, 128], bf16, tag="psT")
                        nc.tensor.transpose(out=pt[:], in_=xb[:], identity=ident[:])
                        nc.vector.tensor_copy(
                            out=x2stage[:, b * 128 : (b + 1) * 128], in_=pt[:])
                    else:
                        xo = sb.tile([128, 64], f32, tag="xo")
                        nc.vector.tensor_scalar_mul(out=xo[:], in0=m1[:], scalar1=0.25)
                        nc.sync.dma_start(out=xout[b * 128 : (b + 1) * 128, :], in_=xo[:])

                if layer == 0:
                    nc.sync.dma_start(out=x2T[0:64, :], in_=x2stage[:])
                    tc.strict_bb_all_engine_barrier()
                    if PROFILE_SINGLE:
                        for c_ in range(C):
                            nc.sync.dma_start(
                                out=ag[c_ * 128 : (c_ + 1) * 128, :], in_=x2T)
                    else:
                        nc.gpsimd.collective_compute(
                            "AllGather", OP.bypass,
                            replica_groups=[list(range(C))],
                            ins=[x2T], outs=[ag],
                        )
                    tc.strict_bb_all_engine_barrier()
                    if DEBUG:
                        nc.sync.dma_start(out=dbg_x2T, in_=x2T)
                        nc.sync.dma_start(out=dbg_ag, in_=ag)

    nc.compile()
    return nc


# ------------------------------------------------------------- entry point
_CACHE = {}


def get_compiled(src, dst, n):
    key = (n, hash(np.asarray(src).tobytes()) ^ hash(np.asarray(dst).tobytes()))
    if key not in _CACHE:
        plan = build_plan(src, dst, n)
        nc = build_program(plan)
        _CACHE[key] = (plan, nc)
    return _CACHE[key]


def make_inputs(plan, x, W1, attn_l1, attn_r1, bias1, W2, attn_l2, attn_r2, bias2):
    n, npos = plan["n"], plan["npos"]
    ntile = -(-npos // 128)
    xTp = np.zeros((128, ntile * 128), np.float32)
    xTp[:64, :n] = np.asarray(x, np.float32).T
    wc1 = np.zeros((128, 320), np.float32)
    wc1[:64, :264] = _fold_weights(W1, attn_l1, attn_r1)
    wc2 = np.zeros((128, 320), np.float32)
    wc2[:64, :264] = _fold_weights(W2, attn_l2, attn_r2)
    pr = _pad_row()
    common = dict(
        xTp=xTp,
        wcat1=wc1,
        wcat2=wc2.astype(BF16),
        bias1r=np.tile(np.asarray(bias1, np.float32)[None, :], (128, 1)),
        bias2r=np.tile(np.asarray(bias2, np.float32)[None, :], (128, 1)),
        padrow=np.stack([pr, pr]),
    )
    in_maps = []
    for c in range(C):
        pc = plan["percore"][c]
        m = dict(common)
        for k in ("gidx1", "gidx2", "eidx1", "eidx2"):
            m[k] = pc[k]
        in_maps.append(m)
    return in_maps


def kernel(x, src, dst, W1, attn_l1, attn_r1, bias1, W2, attn_l2, attn_r2, bias2):
    x = np.asarray(x)
    n = x.shape[0]
    src = np.asarray(src, np.int64)
    dst = np.asarray(dst, np.int64)
    plan, nc = get_compiled(src, dst, n)
    in_maps = make_inputs(plan, x, W1, attn_l1, attn_r1, bias1,
                          W2, attn_l2, attn_r2, bias2)
    res = run_bass_kernel_spmd(nc, in_maps, list(range(C)))
    out = np.empty((n, 64), np.float32)
    for c in range(C):
        xo = res.results[c]["xout"]
        o2 = plan["percore"][c]["order2"]
        real = o2 >= 0
        out[o2[real]] = xo[real]
    return out


# revision 11
# speedup vs baseline: 10.6550x; 1.2919x over previous
"""Trainium2 Bass kernel for a 2-layer GAT (nn_GAT_Module_90623809945643).

Strategy (8 NeuronCores, SPMD):
  - Nodes are partitioned across the 8 cores by global id (6250 each); each
    core owns the edges whose dst it owns, so edge-softmax and segment-sum
    stay local (dst-owner bucketing).
  - Per layer, every core builds the full per-node feature table
    [h (bf16, 256) | el (f32, 4) | er (f32, 4) | pad] (768B rows) from
    replicated x (layer 1) / the all-gathered layer-1 output (layer 2) with
    TensorE matmuls; attn_l/attn_r are folded into the weight matrix on the
    host (el = x @ (W_h @ attn_l_h)), so el/er are matmul outputs.
  - Edge phase: nodes-on-partitions layout. The host sorts each core's nodes
    by in-degree and packs each node's incoming edges into fixed-width slot
    grids (degree bucketing); h[src]/el[src] rows are fetched with dma_gather
    (int16 indices, table split at the 32768-row boundary), er comes
    per-partition via indirect DMA. The weighted segment-sum runs as
    broadcast-exp (ACT) + multiply + pairwise-tree reduction (DVE); the
    softmax division happens once per node after aggregation (the reference's
    max-subtraction cancels exactly and is skipped; scores are O(1) so exp
    cannot overflow).
  - One AllGather (bf16, transposed layer-1 output) between the layers.

The host performs index-only preprocessing (bucketing, padding, index
streams); all floating-point work on x happens on the device.
"""

import sys

sys.path.insert(0, "/opt/trn_rl_repo")

import numpy as np
import ml_dtypes

import concourse.bass as bass
import concourse.bacc as bacc
import concourse.mybir as mybir
import concourse.tile as tile
from concourse.bass_utils import run_bass_kernel_spmd
from concourse.masks import make_identity

BF16 = ml_dtypes.bfloat16

# ---------------- configuration (overridable for small-scale testing) ----
N = 50000          # nodes
H, D = 4, 64       # heads, head dim
NEG = 0.2          # leaky relu slope
C = 8              # cores
KC = 32            # max slot columns per chunk (SBUF bound)
LO_CAP = 32768     # rows in the low table region (int16 index bound)
PAD_EL = -30000.0  # el of the pad row => exp(score) == 0
ROW = 384          # bf16 elems/row: h[0:256] | el f32 @ 256:264 | er f32 @ 264:272 | pad
GRP = 8            # phase-A tiles per group
DEBUG = False      # add intermediate dump outputs
PROFILE_SINGLE = False  # replace collective with local DMAs (TimelineSim profiling)
REPEAT = 1         # repeat the whole 2-layer computation (timing slope)

HD = H * D


def _derived(n):
    npc = n // C
    npad = -(-npc // 128) * 128
    nb = npad // 128
    npos = C * npad
    top = max(npos, n)
    hi = top - (LO_CAP - 1) + 1 if top >= LO_CAP - 1 else 1
    hi_cap = -(-(hi + 1) // 64) * 64
    return npc, npad, nb, npos, hi_cap


# ---------------------------------------------------------------- planning
def _pack_idx(flat):
    """dma_gather index stream: index i lives at [i % 16, i // 16]; the
    16-row group is replicated 8x across 128 partitions."""
    flat = np.asarray(flat, np.int16)
    a = flat.reshape(-1, 16).T
    return np.tile(a, (8, 1))


def _cumcount(key):
    n = len(key)
    starts = np.where(np.r_[True, key[1:] != key[:-1]], np.arange(n), 0)
    return np.arange(n) - np.maximum.accumulate(starts)


def build_plan(src, dst, n=None):
    n = n or N
    npc, npad, nb, npos, hi_cap = _derived(n)
    hi_pad = hi_cap - 1
    lo_pad = LO_CAP - 1
    src = np.asarray(src, np.int64)
    dst = np.asarray(dst, np.int64)
    ecore = dst // npc
    dloc = dst % npc

    layers = []
    percore = [dict() for _ in range(C)]
    rank1 = np.empty(n, np.int64)

    for layer in (0, 1):
        if layer == 0:
            spos = src
        else:
            pos2 = (np.arange(n) // npc) * npad + rank1
            spos = pos2[src]
        islo = (spos <= LO_CAP - 2).astype(np.int64)
        srclo = np.minimum(spos, lo_pad)
        srchi = np.maximum(spos - (LO_CAP - 1), 0)

        deg_lo = np.zeros((C, npc), np.int64)
        deg_hi = np.zeros((C, npc), np.int64)
        for c in range(C):
            m = ecore == c
            np.add.at(deg_lo[c], dloc[m][islo[m] == 1], 1)
            np.add.at(deg_hi[c], dloc[m][islo[m] == 0], 1)

        orders = []
        ranks = np.empty((C, npc), np.int64)
        KLc = np.zeros((C, nb), np.int64)
        KHc = np.zeros((C, nb), np.int64)
        for c in range(C):
            o = np.lexsort((-deg_hi[c], -deg_lo[c]))
            orders.append(o)
            ranks[c, o] = np.arange(npc)
            dl = np.zeros(npad, np.int64)
            dh = np.zeros(npad, np.int64)
            dl[:npc] = deg_lo[c][o]
            dh[:npc] = deg_hi[c][o]
            KLc[c] = dl.reshape(nb, 128).max(1)
            KHc[c] = dh.reshape(nb, 128).max(1)
        KL = KLc.max(0)
        KH = KHc.max(0)

        # shared chunk structure
        blocks = []
        for b in range(nb):
            chunks, cur, cur_w = [], [], 0
            for tab, w in ((0, int(KL[b])), (1, int(KH[b]))):
                while w > 0:
                    take = min(w, KC - cur_w)
                    cur.append([tab, take])
                    cur_w += take
                    w -= take
                    if cur_w == KC:
                        chunks.append(cur)
                        cur, cur_w = [], 0
            if cur:
                chunks.append(cur)
            if not chunks:
                chunks = [[[0, 1]]]
            blocks.append(chunks)

        for c in range(C):
            m = ecore == c
            dr = ranks[c][dloc[m]]
            lo_e = islo[m]
            # per-block slot grids
            grids = []
            for b in range(nb):
                w = int(KL[b] + KH[b])
                if w == 0:
                    w = 1
                g = np.empty((128, w), np.int16)
                g[:, : int(KL[b])] = lo_pad
                g[:, int(KL[b]):] = hi_pad
                if KL[b] + KH[b] == 0:
                    g[:] = lo_pad
                grids.append(g)
            if m.sum() > 0:
                order = np.lexsort((1 - lo_e, dr))
                r_s = dr[order]
                lo_s = lo_e[order]
                j = _cumcount(r_s * 2 + (1 - lo_s))
                srow = np.where(lo_s == 1, srclo[m][order], srchi[m][order])
                b_s = r_s // 128
                p_s = r_s % 128
                for b in range(nb):
                    mb = b_s == b
                    if not mb.any():
                        continue
                    col = np.where(lo_s[mb] == 1, j[mb], KL[b] + j[mb])
                    grids[b][p_s[mb], col] = srow[mb]
            parts = []
            for b in range(nb):
                g = grids[b]
                col0 = 0
                for ch in blocks[b]:
                    for tab, w in ch:
                        sub = g[:, col0 : col0 + w]
                        parts.append(_pack_idx(sub.T.ravel()))
                        col0 += w
            gi = np.concatenate(parts, axis=1)
            glob = np.full(npad, -1, np.int64)
            glob[:npc] = c * npc + orders[c]
            loc = np.maximum(glob - c * npc, 0)
            if layer == 0:
                p_of = glob.copy()
            else:
                p_of = np.where(glob >= 0, c * npad + ranks[c][loc], -1)
            trow = np.where(p_of < 0, lo_pad, p_of + (p_of >= LO_CAP - 1)).astype(np.int32)
            percore[c]["gidx%d" % (layer + 1)] = gi
            percore[c]["eidx%d" % (layer + 1)] = trow.reshape(nb, 128).T.copy()
            if layer == 1:
                percore[c]["order2"] = glob

        if layer == 0:
            for c in range(C):
                rank1[c * npc : (c + 1) * npc] = ranks[c]

        layers.append(dict(KL=KL, KH=KH, blocks=blocks))

    return dict(layers=layers, percore=percore, n=n, npc=npc, npad=npad,
                nb=nb, npos=npos, hi_cap=hi_cap)


# ---------------------------------------------------------- host weights
def _fold_weights(W, attn_l, attn_r):
    W = np.asarray(W, np.float32)
    Wh = W.reshape(64, H, D)
    wl = np.einsum("khd,hd->kh", Wh, np.asarray(attn_l, np.float32))
    wr = np.einsum("khd,hd->kh", Wh, np.asarray(attn_r, np.float32))
    return np.concatenate([W, wl, wr], axis=1)


def _pad_row():
    r = np.zeros(ROW // 2, np.float32)
    r[128:132] = PAD_EL
    return r.view(BF16)


# ------------------------------------------------------------- program
def build_program(plan):
    npad, nb, npos, hi_cap = plan["npad"], plan["nb"], plan["npos"], plan["hi_cap"]
    ntab = LO_CAP + hi_cap
    ntile = -(-npos // 128)
    assert ntile == C * nb and ntile % GRP == 0 and GRP % 2 == 0
    f32, bf16 = mybir.dt.float32, mybir.dt.bfloat16
    i16, i32 = mybir.dt.int16, mybir.dt.int32
    AX = mybir.AxisListType.X
    OP = mybir.AluOpType
    AF = mybir.ActivationFunctionType

    nc = bacc.Bacc("TRN2", target_bir_lowering=False, debug=False, num_devices=C,
                   num_swdge_queues=4)

    xTp = nc.dram_tensor("xTp", [128, ntile * 128], f32, kind="ExternalInput").ap()
    wcat1 = nc.dram_tensor("wcat1", [128, 320], f32, kind="ExternalInput").ap()
    wcat2 = nc.dram_tensor("wcat2", [128, 320], bf16, kind="ExternalInput").ap()
    bias1r = nc.dram_tensor("bias1r", [128, 256], f32, kind="ExternalInput").ap()
    bias2r = nc.dram_tensor("bias2r", [128, 256], f32, kind="ExternalInput").ap()
    padrow = nc.dram_tensor("padrow", [2, ROW], bf16, kind="ExternalInput").ap()
    X1 = plan["percore"][0]["gidx1"].shape[1]
    X2 = plan["percore"][0]["gidx2"].shape[1]
    gidx1 = nc.dram_tensor("gidx1", [128, X1], i16, kind="ExternalInput").ap()
    gidx2 = nc.dram_tensor("gidx2", [128, X2], i16, kind="ExternalInput").ap()
    eidx1 = nc.dram_tensor("eidx1", [128, nb], i32, kind="ExternalInput").ap()
    eidx2 = nc.dram_tensor("eidx2", [128, nb], i32, kind="ExternalInput").ap()
    xout = nc.dram_tensor("xout", [npad, 64], f32, kind="ExternalOutput").ap()

    if DEBUG:
        dbg_t1 = nc.dram_tensor("dbg_t1", [ntab, ROW], bf16, kind="ExternalOutput").ap()
        dbg_t2 = nc.dram_tensor("dbg_t2", [ntab, ROW], bf16, kind="ExternalOutput").ap()
        dbg_er1 = nc.dram_tensor("dbg_er1", [128, 0 + (plan["nb"] * 16)], bf16, kind="ExternalOutput").ap()
        dbg_x2T = nc.dram_tensor("dbg_x2T", [128, npad], bf16, kind="ExternalOutput").ap()
        dbg_ag = nc.dram_tensor("dbg_ag", [C * 128, npad], bf16, kind="ExternalOutput").ap()
    table1 = nc.dram_tensor("table1", [ntab, ROW], bf16).ap()
    table2 = nc.dram_tensor("table2", [ntab, ROW], bf16).ap()
    x2T = nc.dram_tensor("x2T", [128, npad], bf16).ap()
    ag = nc.dram_tensor("agx2", [C * 128, npad], bf16, addr_space="Shared").ap()

    bnd = LO_CAP - 1  # position that maps to the hi region

    with tile.TileContext(nc) as tc:
        with (
            tc.tile_pool(name="const", bufs=1) as constp,
            tc.tile_pool(name="sbuf", bufs=3) as sb,
            tc.tile_pool(name="psum", bufs=2, space="PSUM") as ps,
        ):
            ident = constp.tile([128, 128], bf16)
            make_identity(nc, ident[:])
            zer = constp.tile([64, 2048], bf16)
            nc.gpsimd.memset(zer[:], 0.0)
            for o in range(0, npad, 2048):
                w = min(2048, npad - o)
                nc.sync.dma_start(out=x2T[64:128, o : o + w], in_=zer[:, :w])

            b1t = constp.tile([128, 256], f32)
            nc.sync.dma_start(out=b1t[:], in_=bias1r[:])
            b2t = constp.tile([128, 256], f32)
            nc.sync.dma_start(out=b2t[:], in_=bias2r[:])
            w1t = constp.tile([128, 320], f32)
            nc.sync.dma_start(out=w1t[:], in_=wcat1[:])
            w2t = constp.tile([128, 320], bf16)
            nc.sync.dma_start(out=w2t[:], in_=wcat2[:])

            for _rep in range(REPEAT):
              for layer in (0, 1):
                L = plan["layers"][layer]
                tab = table1 if layer == 0 else table2
                tab_lo = tab[0:LO_CAP]
                tab_hi = tab[LO_CAP:ntab]
                gidx = gidx1 if layer == 0 else gidx2
                eidx = eidx1 if layer == 0 else eidx2
                XX = X1 if layer == 0 else X2
                bias_t = b1t if layer == 0 else b2t
                wt = w1t if layer == 0 else w2t

                # ---------------- phase A: build the table --------------
                for g0 in range(0, ntile, GRP):
                    if layer == 0:
                        lh = sb.tile([128, GRP * 128], f32, tag="lhsA")
                        nc.scalar.dma_start(out=lh[:],
                                            in_=xTp[:, g0 * 128 : (g0 + GRP) * 128])
                    else:
                        lh = sb.tile([128, GRP * 128], bf16, tag="lhsAb")
                        u = g0
                        while u < g0 + GRP:
                            c_ = u // nb
                            j0 = u % nb
                            run = min(g0 + GRP - u, nb - j0)
                            nc.scalar.dma_start(
                                out=lh[:, (u - g0) * 128 : (u - g0 + run) * 128],
                                in_=ag[c_ * 128 : (c_ + 1) * 128,
                                       j0 * 128 : (j0 + run) * 128],
                            )
                            u += run
                    rows = sb.tile([128, GRP, ROW], bf16, tag="rowsA")
                    rowsap = rows[:]
                    rows32 = rowsap.bitcast(f32)
                    for pair in range(GRP // 2):
                        pt = ps.tile([128, 1024], f32, tag="psA")
                        for s in range(2):
                            u = pair * 2 + s
                            nc.tensor.matmul(
                                pt[:, s * 512 : s * 512 + 320],
                                lhsT=lh[:, u * 128 : (u + 1) * 128],
                                rhs=wt[:],
                                start=True, stop=True,
                            )
                        ptap = pt[:]
                        hsrc = bass.AP(ptap.tensor, ptap.offset,
                                       [ptap.ap[0], [512, 2], [1, 256]])
                        hdst = bass.AP(rowsap.tensor, rowsap.offset + pair * 2 * ROW,
                                       [rowsap.ap[0], [ROW, 2], [1, 256]])
                        esrc = bass.AP(ptap.tensor, ptap.offset + 256,
                                       [ptap.ap[0], [512, 2], [1, 64]])
                        edst = bass.AP(rows32.tensor,
                                       rows32.offset + pair * 2 * (ROW // 2) + 128,
                                       [rows32.ap[0], [ROW // 2, 2], [1, 64]])
                        if pair % 2 == 0:
                            nc.vector.tensor_copy(out=hdst, in_=hsrc)
                            nc.vector.tensor_copy(out=edst, in_=esrc)
                        else:
                            nc.scalar.copy(out=hdst, in_=hsrc)
                            nc.scalar.copy(out=edst, in_=esrc)
                    # store rows -> table, skipping over the lo/hi pad row
                    r0 = g0 * 128
                    nrows = GRP * 128
                    if r0 + nrows <= bnd:
                        segs = [(0, nrows, 0)]
                    elif r0 >= bnd:
                        segs = [(0, nrows, 1)]
                    else:
                        segs = [(0, bnd - r0, 0), (bnd - r0, nrows, 1)]
                    for a0, a1, shift in segs:
                        a = a0
                        while a < a1:
                            s0, p0 = divmod(a, 128)
                            dstr = r0 + a + shift
                            if p0 == 0 and a1 - a >= 128:
                                ns = (a1 - a) // 128
                                srcap = bass.AP(
                                    rowsap.tensor, rowsap.offset + s0 * ROW,
                                    [[rowsap.ap[0][0], 128], [ROW, ns], [1, ROW]])
                                dstap = bass.AP(
                                    tab.tensor, tab.offset + dstr * ROW,
                                    [[ROW, 128], [128 * ROW, ns], [1, ROW]])
                                nc.sync.dma_start(out=dstap, in_=srcap)
                                a += ns * 128
                            else:
                                cnt = min(a1 - a, 128 - p0)
                                srcap = bass.AP(
                                    rowsap.tensor,
                                    rowsap.offset + s0 * ROW,
                                    [[rowsap.ap[0][0], 128], [1, ROW]])[p0 : p0 + cnt]
                                nc.sync.dma_start(
                                    out=tab[dstr : dstr + cnt], in_=srcap)
                                a += cnt
                nc.sync.dma_start(out=tab[bnd : bnd + 1], in_=padrow[0:1])
                nc.sync.dma_start(out=tab[ntab - 1 : ntab], in_=padrow[1:2])

                tc.strict_bb_all_engine_barrier()
                if DEBUG:
                    nc.sync.dma_start(out=(dbg_t1 if layer == 0 else dbg_t2), in_=tab)

                # ---------------- edge phase ----------------------------
                idx_sb = constp.tile([128, XX], i16, tag="gidx%d" % layer)
                nc.sync.dma_start(out=idx_sb[:], in_=gidx[:])
                eidx_sb = constp.tile([128, nb], i32, tag="eidx%d" % layer)
                nc.sync.dma_start(out=eidx_sb[:], in_=eidx[:])
                er_sb = sb.tile([128, nb * 16], bf16, tag="er")
                er32 = er_sb[:].bitcast(f32)
                for b in range(nb):
                    nc.gpsimd.indirect_dma_start(
                        out=er_sb[:, b * 16 : (b + 1) * 16],
                        out_offset=None,
                        in_=tab,
                        in_offset=bass.IndirectOffsetOnAxis(
                            ap=eidx_sb[:, b : b + 1], axis=0),
                        element_offset=256,
                    )

                if DEBUG and layer == 0:
                    nc.sync.dma_start(out=dbg_er1, in_=er_sb[:])
                x2stage = (sb.tile([64, nb * 128], bf16, tag="x2s", name="x2stage")
                           if layer == 0 else None)
                icol = 0
                gq = 0
                for b in range(nb):
                    chunks = L["blocks"][b]
                    num = None     # (tensor, offset, pstride) of a [128,256] value
                    zacc = None
                    for ci, ch in enumerate(chunks):
                        w = sum(x[1] for x in ch)
                        G = sb.tile([128, KC, ROW], bf16, tag="G")
                        c0 = 0
                        for tabi, cw in ch:
                            nidx = 128 * cw
                            nc.gpsimd.dma_gather(
                                G[:, c0 : c0 + cw, :],
                                tab_lo if tabi == 0 else tab_hi,
                                idx_sb[:, icol : icol + nidx // 16],
                                nidx, nidx, ROW,
                                single_packet=False, queue_num=gq % 4,
                            )
                            gq += 1
                            icol += nidx // 16
                            c0 += cw
                        Ga = G[:]
                        elf = Ga[:, :w, 256:272].bitcast(f32)[:, :, 0:4]
                        erb = bass.AP(er32.tensor, er32.offset + b * 8 + 4,
                                      [er32.ap[0], [0, w], [1, 4]])
                        S = sb.tile([128, KC * 4], f32, tag="S")
                        S3 = S[:, : w * 4].rearrange("p (k f) -> p k f", f=4)
                        nc.vector.tensor_tensor(out=S3, in0=elf, in1=erb, op=OP.add)
                        T = sb.tile([128, KC * 4], f32, tag="T")
                        nc.vector.tensor_scalar_mul(
                            out=T[:, : w * 4], in0=S[:, : w * 4], scalar1=NEG)
                        nc.vector.tensor_tensor(
                            out=S[:, : w * 4], in0=S[:, : w * 4],
                            in1=T[:, : w * 4], op=OP.max)
                        P4 = sb.tile([128, KC * 4], f32, tag="P4")
                        nc.scalar.activation(
                            out=P4[:, : w * 4], in_=S[:, : w * 4], func=AF.Exp)
                        zc = sb.tile([128, 4], f32, tag="zc")
                        p4ap = P4[:]
                        p4perm = bass.AP(p4ap.tensor, p4ap.offset,
                                         [p4ap.ap[0], [1, 4], [4, w]])
                        nc.vector.tensor_reduce(out=zc[:], in_=p4perm, axis=AX, op=OP.add)
                        PE = sb.tile([128, KC, 256], bf16, tag="PE")
                        sap = S[:]
                        sbc = bass.AP(sap.tensor, sap.offset,
                                      [sap.ap[0], [4, w], [1, 4], [0, 64]])
                        pe4 = PE[:, :w, :].rearrange("p k (h d) -> p k h d", d=64)
                        nc.scalar.activation(out=pe4, in_=sbc, func=AF.Exp)
                        nc.vector.tensor_tensor(
                            out=PE[:, :w, :], in0=Ga[:, :w, 0:256],
                            in1=PE[:, :w, :], op=OP.mult)
                        k = w
                        while k > 1:
                            hh = k // 2
                            ce = k - hh
                            nc.vector.tensor_tensor(
                                out=PE[:, :hh, :], in0=PE[:, :hh, :],
                                in1=PE[:, ce : ce + hh, :], op=OP.add)
                            k = ce
                        peap = PE[:]
                        if ci == 0:
                            num = (peap.tensor, peap.offset, peap.ap[0])
                            zacc = zc
                        else:
                            nu = sb.tile([128, 256], f32, tag="nacc")
                            prev = bass.AP(num[0], num[1], [num[2], [1, 256]])
                            cur = bass.AP(peap.tensor, peap.offset, [peap.ap[0], [1, 256]])
                            nc.vector.tensor_tensor(out=nu[:], in0=prev, in1=cur, op=OP.add)
                            nuap = nu[:]
                            num = (nuap.tensor, nuap.offset, nuap.ap[0])
                            nc.vector.tensor_tensor(out=zacc[:], in0=zacc[:], in1=zc[:], op=OP.add)
                    # ---- node epilogue
                    nc.vector.tensor_scalar_max(out=zacc[:], in0=zacc[:], scalar1=1e-30)
                    rz = sb.tile([128, 4], f32, tag="rz")
                    nc.vector.reciprocal(rz[:], zacc[:])
                    rzap = rz[:]
                    rzb = bass.AP(rzap.tensor, rzap.offset,
                                  [rzap.ap[0], [1, 4], [0, 64]])
                    numv = bass.AP(num[0], num[1], [num[2], [64, 4], [1, 64]])
                    O = sb.tile([128, 256], f32, tag="O")
                    nc.vector.tensor_tensor(
                        out=O[:].rearrange("p (h d) -> p h d", d=64),
                        in0=numv, in1=rzb, op=OP.mult)
                    nc.vector.tensor_tensor(out=O[:], in0=O[:], in1=bias_t[:], op=OP.add)
                    if layer == 0:
                        O2 = sb.tile([128, 256], f32, tag="O2")
                        nc.scalar.activation(out=O2[:], in_=O[:], func=AF.Tanh)
                    else:
                        O2 = O
                    m1 = sb.tile([128, 64], f32, tag="m1")
                    m2 = sb.tile([128, 64], f32, tag="m2")
                    nc.vector.tensor_tensor(out=m1[:], in0=O2[:, 0:64],
                                            in1=O2[:, 64:128], op=OP.add)
                    nc.vector.tensor_tensor(out=m2[:], in0=O2[:, 128:192],
                                            in1=O2[:, 192:256], op=OP.add)
                    nc.vector.tensor_tensor(out=m1[:], in0=m1[:], in1=m2[:], op=OP.add)
                    if layer == 0:
                        xb = sb.tile([128, 64], bf16, tag="xb")
                        nc.vector.tensor_scalar_mul(out=xb[:], in0=m1[:], scalar1=0.25)
                        pt = ps.tile([64, 128], bf16, tag="psT")
                        nc.tensor.transpose(out=pt[:], in_=xb[:], identity=ident[:])
                        nc.vector.tensor_copy(
                            out=x2stage[:, b * 128 : (b + 1) * 128], in_=pt[:])
                    else:
                        xo = sb.tile([128, 64], f32, tag="xo")
                        nc.vector.tensor_scalar_mul(out=xo[:], in0=m1[:], scalar1=0.25)
                        nc.sync.dma_start(out=xout[b * 128 : (b + 1) * 128, :], in_=xo[:])

                if layer == 0:
                    nc.sync.dma_start(out=x2T[0:64, :], in_=x2stage[:])
                    tc.strict_bb_all_engine_barrier()
                    if PROFILE_SINGLE:
                        for c_ in range(C):
                            nc.sync.dma_start(
                                out=ag[c_ * 128 : (c_ + 1) * 128, :], in_=x2T)
                    else:
                        nc.gpsimd.collective_compute(
                            "AllGather", OP.bypass,
                            replica_groups=[list(range(C))],
                            ins=[x2T], outs=[ag],
                        )
                    tc.strict_bb_all_engine_barrier()
                    if DEBUG:
                        nc.sync.dma_start(out=dbg_x2T, in_=x2T)
                        nc.sync.dma_start(out=dbg_ag, in_=ag)

    nc.compile()
    return nc


# ------------------------------------------------------------- entry point
_CACHE = {}


def get_compiled(src, dst, n):
    key = (n, hash(np.asarray(src).tobytes()) ^ hash(np.asarray(dst).tobytes()))
    if key not in _CACHE:
        plan = build_plan(src, dst, n)
        nc = build_program(plan)
        _CACHE[key] = (plan, nc)
    return _CACHE[key]


def make_inputs(plan, x, W1, attn_l1, attn_r1, bias1, W2, attn_l2, attn_r2, bias2):
    n, npos = plan["n"], plan["npos"]
    ntile = -(-npos // 128)
    xTp = np.zeros((128, ntile * 128), np.float32)
    xTp[:64, :n] = np.asarray(x, np.float32).T
    wc1 = np.zeros((128, 320), np.float32)
    wc1[:64, :264] = _fold_weights(W1, attn_l1, attn_r1)
    wc2 = np.zeros((128, 320), np.float32)
    wc2[:64, :264] = _fold_weights(W2, attn_l2, attn_r2)
    pr = _pad_row()
    common = dict(
        xTp=xTp,
        wcat1=wc1,
        wcat2=wc2.astype(BF16),
        bias1r=np.tile(np.asarray(bias1, np.float32)[None, :], (128, 1)),
        bias2r=np.tile(np.asarray(bias2, np.float32)[None, :], (128, 1)),
        padrow=np.stack([pr, pr]),
    )
    in_maps = []
    for c in range(C):
        pc = plan["percore"][c]
        m = dict(common)
        for k in ("gidx1", "gidx2", "eidx1", "eidx2"):
            m[k] = pc[k]
        in_maps.append(m)
    return in_maps


def kernel(x, src, dst, W1, attn_l1, attn_r1, bias1, W2, attn_l2, attn_r2, bias2):
    x = np.asarray(x)
    n = x.shape[0]
    src = np.asarray(src, np.int64)
    dst = np.asarray(dst, np.int64)
    plan, nc = get_compiled(src, dst, n)
    in_maps = make_inputs(plan, x, W1, attn_l1, attn_r1, bias1,
                          W2, attn_l2, attn_r2, bias2)
    res = run_bass_kernel_spmd(nc, in_maps, list(range(C)))
    out = np.empty((n, 64), np.float32)
    for c in range(C):
        xo = res.results[c]["xout"]
        o2 = plan["percore"][c]["order2"]
        real = o2 >= 0
        out[o2[real]] = xo[real]
    return out
